# revision 40
# baseline (speedup 1.0000x reference)
"""BaseCrossAttention Trainium2 kernel (v3 — software-pipelined emission).

Full inputs -> full output. Shards batch B=32 across 8 NeuronCores (4 each),
builds one SPMD Bass/Tile program, runs via run_bass_kernel_spmd.

Math notes (exact rewrites of the reference):
 - softmax over N is shift-invariant, so the uniform (1-tcond)*-1e6 mask term
   cancels; gating is fully carried by v *= tcond.
 - the stylization scale/shift (silu(emb) @ We + be) depends only on per-batch
   emb, so it is precomputed on the host and folded with ln_y_g/ln_y_b into a
   per-(batch, d) affine A, C:  h = silu(A * ynorm + C) @ Wo + bo.
 - attention runs "transposed" (feature dim on partitions, tokens free); the
   softmax denominator is computed unnormalized via masked-ones matmuls.

Engine/queue plan (avoids sequencer head-of-line blocking):
 - Pool/gpsimd (SWDGE): all DRAM loads (with f32->bf16 cast) + final store
   (bf16->f32 cast), next batch prefetched ahead of compute.
 - SP (HWDGE): input-side 128x1024 batched XBAR transposes only.
 - ACT: normalizations, exp, silu, Wo-psum drains + output-side transposes.
 - DVE: bn_stats, psum drains, reciprocal, yc, stylization muls, residual add.
 - PE: all matmuls, two heads packed per 128-row band (row/col tile packing).
Emission is software-pipelined: batch b+1's layernorms and K/V projection are
emitted between batch b's attention and its Wo matmuls, so the in-order PE
queue always has ready work while the LN(y)->silu serial chain resolves.
"""

import sys

sys.path.insert(0, "/opt/trn_rl_repo")

from contextlib import ExitStack

import numpy as np

import concourse.bass as bass
import concourse.tile as tile
from concourse import bacc, mybir
from concourse.bass_utils import run_bass_kernel_spmd

F32 = mybir.dt.float32
BF16 = mybir.dt.bfloat16
AF = mybir.ActivationFunctionType
ALU = mybir.AluOpType

B, T, D = 32, 1024, 1024
N, L = 256, 768
H, HD, TE = 16, 64, 2048
NCORES = 8
BPC = B // NCORES
P = 128
TC = T // P   # 8 t-chunks
DC = D // P   # 8 d-chunks
LC = L // P   # 6 l-chunks
NC2 = N // P  # 2 n-chunks
TH = T // 512  # 2 t-halves per PSUM pair

LAST_RESULT = None
LAST_NC = None

# fp8 (e4m3 + DoubleRow) for the two big GEMMs; verified against the 2e-2 gate
USE_FP8_Q = False
USE_FP8_WO = False
F8 = mybir.dt.float8e4
PM_DR = mybir.MatmulPerfMode.DoubleRow


def build(nc, flags):
    d = {}
    d["x"] = nc.dram_tensor("x", [BPC, T, D], F32, kind="ExternalInput").ap()
    d["xf"] = nc.dram_tensor("xf", [BPC, N, L], F32, kind="ExternalInput").ap()
    d["gate"] = nc.dram_tensor("gate", [BPC, 1], F32, kind="ExternalInput").ap()
    d["Asty"] = nc.dram_tensor("Asty", [BPC, D], F32, kind="ExternalInput").ap()
    d["Csty"] = nc.dram_tensor("Csty", [BPC, D], F32, kind="ExternalInput").ap()
    for nm, shp in [("Wq", [D, D]), ("Wk", [L, D]), ("Wv", [L, D]),
                    ("Wo", [D, D])]:
        d[nm] = nc.dram_tensor(nm, shp, F32, kind="ExternalInput").ap()
    for nm, n in [("bq", D), ("bk", D), ("bv", D), ("bo", D),
                  ("ln_x_g", D), ("ln_x_b", D), ("ln_t_g", L), ("ln_t_b", L)]:
        d[nm] = nc.dram_tensor(nm, [n], F32, kind="ExternalInput").ap()
    out = nc.dram_tensor("out", [BPC, T, D], F32, kind="ExternalOutput").ap()

    with tile.TileContext(nc) as tc, ExitStack() as ctx:
        consts = ctx.enter_context(tc.tile_pool(name="consts", bufs=1))
        xst_p = ctx.enter_context(tc.tile_pool(name="xst", bufs=2))
        xfst_p = ctx.enter_context(tc.tile_pool(name="xfst", bufs=1))
        lnxc_p = ctx.enter_context(tc.tile_pool(name="lnxc", bufs=2))
        lnxT_p = ctx.enter_context(tc.tile_pool(name="lnxT", bufs=1))
        xfnT_p = ctx.enter_context(tc.tile_pool(name="xfnT", bufs=1))
        kT_p = ctx.enter_context(tc.tile_pool(name="kT", bufs=1))
        vv_p = ctx.enter_context(tc.tile_pool(name="vv", bufs=1))
        qT_p = ctx.enter_context(tc.tile_pool(name="qT", bufs=3))
        pT_p = ctx.enter_context(tc.tile_pool(name="pT", bufs=2))
        rs_p = ctx.enter_context(tc.tile_pool(name="rs", bufs=2))
        big_p = ctx.enter_context(tc.tile_pool(name="big", bufs=2))
        if USE_FP8_WO:
            sil_p = ctx.enter_context(tc.tile_pool(name="sil", bufs=1))
        if USE_FP8_Q:
            lnx8_p = ctx.enter_context(tc.tile_pool(name="lnx8", bufs=1))
        st_p = ctx.enter_context(tc.tile_pool(name="st", bufs=1))
        sm_p = ctx.enter_context(tc.tile_pool(name="sm", bufs=2))
        hc_p = ctx.enter_context(tc.tile_pool(name="hc", bufs=2))
        ps2 = ctx.enter_context(tc.tile_pool(name="ps2", bufs=2, space="PSUM"))
        pa_p = ctx.enter_context(tc.tile_pool(name="pa", bufs=2, space="PSUM"))

        # ---------- constants ----------
        ones = consts.tile([P, P], BF16)
        nc.vector.memset(ones, 1.0)
        ones_top = consts.tile([P, P], BF16)
        nc.vector.memset(ones_top, 0.0)
        nc.vector.memset(ones_top[:, 0:64], 1.0)
        ones_bot = consts.tile([P, P], BF16)
        nc.vector.memset(ones_bot, 0.0)
        nc.vector.memset(ones_bot[:, 64:128], 1.0)
        eps_c = consts.tile([P, 1], F32)
        nc.vector.memset(eps_c, 1e-5)

        # ---------- batch input loads (Pool queue, casts f32->bf16) ----------
        xst = {}
        xfst = {}

        def load_batch(b):
            # xf first: the K/V projection is the earliest PE work of a batch
            ft = xfst_p.tile([P, NC2, L], BF16, tag="xfst", name=f"xfst{b}")
            nc.gpsimd.dma_start(
                out=ft[:], in_=d["xf"][b].rearrange("(c p) d -> p c d", p=P))
            xfst[b] = ft
            xt = xst_p.tile([P, TC, D], BF16, tag="xst", name=f"xst{b}")
            for hh in range(2):
                nc.gpsimd.dma_start(
                    out=xt[:, hh * 4:(hh + 1) * 4, :],
                    in_=d["x"][b, hh * 512:(hh + 1) * 512, :]
                    .rearrange("(c p) d -> p c d", p=P))
            xst[b] = xt

        # cold start: xf + Wk first (earliest PE work), then x, then the rest
        ft0 = xfst_p.tile([P, NC2, L], BF16, tag="xfst", name="xfst0")
        nc.gpsimd.dma_start(
            out=ft0[:], in_=d["xf"][0].rearrange("(c p) d -> p c d", p=P))
        xfst[0] = ft0
        Wk_bf = consts.tile([P, LC, D], BF16)
        nc.gpsimd.dma_start(out=Wk_bf[:],
                            in_=d["Wk"].rearrange("(c p) d -> p c d", p=P))
        xt0 = xst_p.tile([P, TC, D], BF16, tag="xst", name="xst0")
        for hh in range(2):
            nc.gpsimd.dma_start(
                out=xt0[:, hh * 4:(hh + 1) * 4, :],
                in_=d["x"][0, hh * 512:(hh + 1) * 512, :]
                .rearrange("(c p) d -> p c d", p=P))
        xst[0] = xt0
        Wv_bf = consts.tile([P, LC, D], BF16)
        nc.gpsimd.dma_start(out=Wv_bf[:],
                            in_=d["Wv"].rearrange("(c p) d -> p c d", p=P))
        if USE_FP8_Q:
            WqS = big_p.tile([P, DC, D], BF16, tag="big", name="WqStage")
            nc.gpsimd.dma_start(out=WqS[:],
                                in_=d["Wq"].rearrange("(c p) d -> p c d", p=P))
            Wq_bf = consts.tile([P, DC, D], F8, name="Wq_f8")
            with nc.allow_low_precision(reason="Wq fp8"):
                for c in range(DC):
                    nc.vector.tensor_copy(out=Wq_bf[:, c, :], in_=WqS[:, c, :])
        else:
            Wq_bf = consts.tile([P, DC, D], BF16)
            nc.gpsimd.dma_start(out=Wq_bf[:],
                                in_=d["Wq"].rearrange("(c p) d -> p c d", p=P))
        if USE_FP8_WO:
            WoS = big_p.tile([P, DC, D], BF16, tag="big", name="WoStage")
            nc.gpsimd.dma_start(out=WoS[:],
                                in_=d["Wo"].rearrange("(c p) d -> p c d", p=P))
            Wo_bf = consts.tile([P, DC, D], F8, name="Wo_f8")
            with nc.allow_low_precision(reason="Wo fp8"):
                for c in range(DC):
                    nc.gpsimd.tensor_copy(out=Wo_bf[:, c, :], in_=WoS[:, c, :])
        else:
            Wo_bf = consts.tile([P, DC, D], BF16)
            nc.gpsimd.dma_start(out=Wo_bf[:],
                                in_=d["Wo"].rearrange("(c p) d -> p c d", p=P))

        # ---------- per-batch scalars (tiny, SP queue; DMAs emitted later
        # so they don't head-of-line block batch 0's transposes) ----------
        gate_t = consts.tile([P, BPC], F32)
        At = consts.tile([P, DC, BPC], F32)
        Ct = consts.tile([P, DC, BPC], F32)

        def emit_const_loads():
            for b in range(BPC):
                gb = d["gate"][b:b + 1, :]
                nc.sync.dma_start(
                    out=gate_t[:, b:b + 1],
                    in_=bass.AP(tensor=gb.tensor, offset=gb.offset,
                                ap=[[0, P], [1, 1]]))
            for b in range(BPC):
                nc.sync.dma_start(
                    out=At[:, :, b],
                    in_=d["Asty"][b, :].rearrange("(c p) -> p c", p=P))
                nc.sync.dma_start(
                    out=Ct[:, :, b],
                    in_=d["Csty"][b, :].rearrange("(c p) -> p c", p=P))

        def vec_pc(name, nchunk):
            t = consts.tile([P, nchunk], F32, tag="v_" + name, name="v_" + name)
            nc.sync.dma_start(out=t[:], in_=d[name].rearrange("(c p) -> p c", p=P))
            return t

        bq_t = vec_pc("bq", DC) if flags["bq"] else None
        bk_t = vec_pc("bk", DC) if flags["bk"] else None
        gx_t = vec_pc("ln_x_g", DC) if flags["bx"] else None
        bx_t = vec_pc("ln_x_b", DC) if flags["bx"] else None
        gt_t = vec_pc("ln_t_g", LC) if flags["bt"] else None
        bt_t = vec_pc("ln_t_b", LC) if flags["bt"] else None
        bo_t = vec_pc("bo", DC)
        if flags["bv"]:
            bv_bc = consts.tile([P, D], F32)
            nc.sync.dma_start(
                out=bv_bc[:],
                in_=bass.AP(tensor=d["bv"].tensor, offset=d["bv"].offset,
                            ap=[[0, P]] + list(d["bv"].ap)))

        # per-batch tile handles threaded between phases
        S = {}

        def ln_chunk(src, width, nsub, sub, dst, b, lbl):
            stats = sm_p.tile([P, nsub, 6], F32, tag=f"st{nsub}", name=f"s{lbl}")
            for i in range(nsub):
                nc.vector.bn_stats(out=stats[:, i, :],
                                   in_=src[:, i * sub:(i + 1) * sub])
            mv = sm_p.tile([P, 2], F32, tag="mv", name=f"mv{lbl}")
            nc.vector.bn_aggr(out=mv[:], in_=stats[:])
            sd1 = sm_p.tile([P, 1], F32, tag="sd1", name=f"sd{lbl}")
            nc.scalar.activation(out=sd1, in_=mv[:, 1:2], func=AF.Sqrt,
                                 bias=eps_c, scale=1.0)
            r1 = sm_p.tile([P, 1], F32, tag="r1", name=f"r1{lbl}")
            nc.vector.reciprocal(out=r1, in_=sd1)
            nmr = sm_p.tile([P, 1], F32, tag="nmr", name=f"nm{lbl}")
            nc.vector.tensor_scalar(out=nmr, in0=mv[:, 0:1], scalar1=r1,
                                    scalar2=-1.0, op0=ALU.mult, op1=ALU.mult)
            nc.scalar.activation(out=dst, in_=src, func=AF.Identity,
                                 bias=nmr, scale=r1)

        def phase_ln(b):
            # LN(xf) first (gates the earliest PE work), then LN(x)
            xfnT = xfnT_p.tile([P, LC, N], BF16, tag="xfnT", name=f"xfnT{b}")
            for nb in range(NC2):
                xfc = lnxc_p.tile([P, L], BF16, tag="xfc", name=f"xfc{b}_{nb}")
                ln_chunk(xfst[b][:, nb, :], L, 3, 256, xfc[:], b, f"f{b}_{nb}")
                nc.sync.dma_start_transpose(
                    out=xfnT[:, :, nb * P:(nb + 1) * P], in_=xfc[:])
            if flags["bt"]:
                for lc in range(LC):
                    nc.vector.tensor_scalar(
                        out=xfnT[:, lc, :], in0=xfnT[:, lc, :],
                        scalar1=gt_t[:, lc:lc + 1], scalar2=bt_t[:, lc:lc + 1],
                        op0=ALU.mult, op1=ALU.add)
            lnxT = lnxT_p.tile([P, DC, T], BF16, tag="lnxT", name=f"lnxT{b}")
            if USE_FP8_Q:
                lnx8 = lnx8_p.tile([P, DC, T], F8, tag="lnx8", name=f"lnx8{b}")
                S[b, "lnx8"] = lnx8
            for tcn in range(TC):
                lnxc = lnxc_p.tile([P, D], BF16, tag="lnxc", name=f"lnxc{b}_{tcn}")
                ln_chunk(xst[b][:, tcn, :], D, 2, 512, lnxc[:], b, f"x{b}_{tcn}")
                nc.sync.dma_start_transpose(
                    out=lnxT[:, :, tcn * P:(tcn + 1) * P], in_=lnxc[:])
                if USE_FP8_Q and not flags["bx"]:
                    with nc.allow_low_precision(reason="lnx fp8"):
                        nc.gpsimd.tensor_copy(
                            out=lnx8[:, :, tcn * P:(tcn + 1) * P],
                            in_=lnxT[:, :, tcn * P:(tcn + 1) * P])
            if flags["bx"]:
                for dc in range(DC):
                    nc.vector.tensor_scalar(
                        out=lnxT[:, dc, :], in0=lnxT[:, dc, :],
                        scalar1=gx_t[:, dc:dc + 1], scalar2=bx_t[:, dc:dc + 1],
                        op0=ALU.mult, op1=ALU.add)
                if USE_FP8_Q:
                    with nc.allow_low_precision(reason="lnx fp8"):
                        for dc in range(DC):
                            nc.gpsimd.tensor_copy(out=lnx8[:, dc, :],
                                                  in_=lnxT[:, dc, :])
            S[b, "xfnT"], S[b, "lnxT"] = xfnT, lnxT

        def phase_kv(b):
            xfnT = S[b, "xfnT"]
            kT = kT_p.tile([P, DC, N], BF16, tag="kT", name=f"kT{b}")
            for mq in range(0, DC, 4):
                pk = ps2.tile([P, 1024], F32, tag="ps2", name=f"pk{b}_{mq}")
                for mo in range(4):
                    m = mq + mo
                    for lc in range(LC):
                        nc.tensor.matmul(
                            pk[:, mo * N:(mo + 1) * N],
                            lhsT=Wk_bf[:, lc, m * P:(m + 1) * P],
                            rhs=xfnT[:, lc, :],
                            start=(lc == 0), stop=(lc == LC - 1))
                if flags["bk"]:
                    for mo in range(4):
                        nc.vector.tensor_scalar_add(
                            out=kT[:, mq + mo, :], in0=pk[:, mo * N:(mo + 1) * N],
                            scalar1=bk_t[:, mq + mo:mq + mo + 1])
                else:
                    nc.vector.tensor_copy(out=kT[:, mq:mq + 4, :], in_=pk[:])
            vv = vv_p.tile([P, NC2, D], BF16, tag="vv", name=f"vv{b}")
            for nb in range(NC2):
                pv = ps2.tile([P, 1024], F32, tag="ps2", name=f"pv{b}_{nb}")
                for dh in range(2):
                    for lc in range(LC):
                        nc.tensor.matmul(
                            pv[:, dh * 512:(dh + 1) * 512],
                            lhsT=xfnT[:, lc, nb * P:(nb + 1) * P],
                            rhs=Wv_bf[:, lc, dh * 512:(dh + 1) * 512],
                            start=(lc == 0), stop=(lc == LC - 1))
                if flags["bv"]:
                    nc.vector.tensor_tensor(out=pv[:], in0=pv[:], in1=bv_bc[:],
                                            op=ALU.add)
                nc.vector.tensor_scalar_mul(out=vv[:, nb, :], in0=pv[:],
                                            scalar1=gate_t[:, b:b + 1])
            S[b, "kT"], S[b, "vv"] = kT, vv

        def phase_attn(b):
            lnxT, kT, vv = S[b, "lnxT"], S[b, "kT"], S[b, "vv"]
            lnx8 = S.get((b, "lnx8"))
            yT = big_p.tile([P, DC, T], BF16, tag="big", name=f"yT{b}")
            qThs, pTs, rss = {}, {}, {}

            def emit_q(hp):
                pq = ps2.tile([P, 1024], F32, tag="ps2", name=f"pq{b}_{hp}")
                if USE_FP8_Q:
                    for th in range(TH):
                        for kc in range(DC // 2):
                            nc.tensor.matmul(
                                pq[:, th * 512:(th + 1) * 512],
                                lhsT=Wq_bf[:, 2 * kc:2 * kc + 2,
                                           hp * P:(hp + 1) * P],
                                rhs=lnx8[:, 2 * kc:2 * kc + 2,
                                         th * 512:(th + 1) * 512],
                                start=(kc == 0), stop=(kc == DC // 2 - 1),
                                perf_mode=PM_DR)
                else:
                    for th in range(TH):
                        for k in range(DC):
                            nc.tensor.matmul(
                                pq[:, th * 512:(th + 1) * 512],
                                lhsT=Wq_bf[:, k, hp * P:(hp + 1) * P],
                                rhs=lnxT[:, k, th * 512:(th + 1) * 512],
                                start=(k == 0), stop=(k == DC - 1))
                qTh = qT_p.tile([P, T], BF16, tag="qTh", name=f"qTh{b}_{hp}")
                if flags["bq"]:
                    nc.vector.tensor_scalar_add(out=qTh[:], in0=pq[:],
                                                scalar1=bq_t[:, hp:hp + 1])
                else:
                    nc.vector.tensor_copy(out=qTh[:], in_=pq[:])
                qThs[hp] = qTh

            def emit_qk_exp(hp):
                qTh = qThs[hp]
                pT = pT_p.tile([P, 2, NC2, T], BF16, tag="pT", name=f"pT{b}_{hp}")
                for th in range(TH):
                    for nb in range(NC2):
                        pa = pa_p.tile([P, 1024], F32, tag="pa",
                                       name=f"pa{b}_{hp}_{th}_{nb}")
                        for j in range(2):
                            r0, r1b = 64 * j, 64 * j + 64
                            nc.tensor.matmul(
                                pa[:, j * 512:(j + 1) * 512],
                                lhsT=kT[r0:r1b, hp, nb * P:(nb + 1) * P],
                                rhs=qTh[r0:r1b, th * 512:(th + 1) * 512],
                                start=True, stop=True)
                        nc.scalar.activation(
                            out=pT[:, :, nb, th * 512:(th + 1) * 512],
                            in_=pa[:].rearrange("p (j t) -> p j t", j=2),
                            func=AF.Exp)
                pTs[hp] = pT

            def emit_sden(hp):
                pT = pTs[hp]
                sps = ps2.tile([P, 1024], F32, tag="ps2", name=f"sps{b}_{hp}")
                for th in range(TH):
                    for j in range(2):
                        for nb in range(NC2):
                            nc.tensor.matmul(
                                sps[:, th * 512:(th + 1) * 512],
                                lhsT=(ones_top if j == 0 else ones_bot)[:],
                                rhs=pT[:, j, nb, th * 512:(th + 1) * 512],
                                start=(j == 0 and nb == 0),
                                stop=(j == 1 and nb == NC2 - 1))
                rs = rs_p.tile([P, T], BF16, tag="rs", name=f"rs{b}_{hp}")
                with nc.allow_low_precision(reason="softmax denom bf16"):
                    nc.vector.reciprocal(out=rs[:], in_=sps[:])
                rss[hp] = rs

            def emit_av(hp):
                pT = pTs[hp]
                pyy = ps2.tile([P, 1024], F32, tag="ps2", name=f"pyy{b}_{hp}")
                for th in range(TH):
                    for nb in range(NC2):
                        for j in range(2):
                            h = 2 * hp + j
                            nc.tensor.matmul(
                                pyy[64 * j:64 * j + 64,
                                    th * 512:(th + 1) * 512],
                                lhsT=vv[:, nb, h * HD:(h + 1) * HD],
                                rhs=pT[:, j, nb, th * 512:(th + 1) * 512],
                                start=(nb == 0), stop=(nb == NC2 - 1),
                                tile_position=(0, 64 * j))
                with nc.allow_low_precision(reason="y bf16"):
                    nc.vector.tensor_tensor(out=yT[:, hp, :], in0=pyy[:],
                                            in1=rss[hp][:], op=ALU.mult)

            emit_q(0)
            emit_qk_exp(0)
            emit_q(1)
            for hp in range(DC):
                if hp + 1 < DC:
                    emit_qk_exp(hp + 1)
                if hp + 2 < DC:
                    emit_q(hp + 2)
                emit_sden(hp)
                emit_av(hp)
            S[b, "yT"] = yT

        def phase_stats(b):
            yT = S[b, "yT"]
            pss = ps2.tile([P, 1024], F32, tag="ps2", name=f"pss{b}")
            psq = ps2.tile([P, 1024], F32, tag="ps2", name=f"psq{b}")
            for dc in range(DC):
                y2 = sm_p.tile([P, T], BF16, tag="y2", name=f"y2{b}_{dc}")
                nc.vector.tensor_mul(out=y2[:], in0=yT[:, dc, :], in1=yT[:, dc, :])
                for th in range(TH):
                    nc.tensor.matmul(pss[:, th * 512:(th + 1) * 512],
                                     lhsT=ones[:],
                                     rhs=yT[:, dc, th * 512:(th + 1) * 512],
                                     start=(dc == 0), stop=(dc == DC - 1))
                    nc.tensor.matmul(psq[:, th * 512:(th + 1) * 512],
                                     lhsT=ones[:],
                                     rhs=y2[:, th * 512:(th + 1) * 512],
                                     start=(dc == 0), stop=(dc == DC - 1))
            sums = st_p.tile([P, T], BF16, tag="sums", name=f"sums{b}")
            sqs = st_p.tile([P, T], BF16, tag="sqs", name=f"sqs{b}")
            m2 = st_p.tile([P, T], BF16, tag="m2", name=f"m2{b}")
            with nc.allow_low_precision(reason="LN(y) stats bf16"):
                nc.vector.tensor_scalar_mul(out=sums[:], in0=pss[:], scalar1=1.0 / D)
                nc.vector.tensor_scalar_mul(out=sqs[:], in0=psq[:], scalar1=1.0 / D)
            nc.vector.tensor_mul(out=m2[:], in0=sums[:], in1=sums[:])
            nc.vector.tensor_sub(out=sqs[:], in0=sqs[:], in1=m2[:])
            nc.scalar.activation(out=m2[:], in_=sqs[:], func=AF.Sqrt,
                                 bias=eps_c, scale=1.0)
            r_bf = st_p.tile([P, T], BF16, tag="r_bf", name=f"r_bf{b}")
            with nc.allow_low_precision(reason="rstd bf16"):
                nc.vector.reciprocal(out=r_bf[:], in_=m2[:])
            mr_bf = st_p.tile([P, T], BF16, tag="mr_bf", name=f"mr_bf{b}")
            with nc.allow_low_precision(reason="mean*rstd bf16"):
                nc.vector.tensor_mul(out=mr_bf[:], in0=sums[:], in1=r_bf[:])
            S[b, "r_bf"], S[b, "mr_bf"] = r_bf, mr_bf

        def phase_zsilu(b):
            yT, r_bf, mr_bf = S[b, "yT"], S[b, "r_bf"], S[b, "mr_bf"]
            silus = (sil_p.tile([P, DC, T], F8, tag="sil", name=f"sil{b}")
                     if USE_FP8_WO else
                     big_p.tile([P, DC, T], BF16, tag="big", name=f"sil{b}"))
            for dc in range(DC):
                zc = sm_p.tile([P, T], BF16, tag="zc", name=f"zc{b}_{dc}")
                nc.vector.tensor_mul(out=zc[:], in0=yT[:, dc, :], in1=r_bf[:])
                nc.vector.tensor_sub(out=zc[:], in0=zc[:], in1=mr_bf[:])
                nc.scalar.activation(out=silus[:, dc, :], in_=zc[:], func=AF.Silu,
                                     bias=Ct[:, dc, b:b + 1], scale=At[:, dc, b:b + 1])
            S[b, "silus"] = silus

        def phase_wo(b):
            silus = S[b, "silus"]
            hTt = big_p.tile([P, TC, D], BF16, tag="big", name=f"hTt{b}")
            for m in range(DC):
                ph = ps2.tile([P, 1024], F32, tag="ps2", name=f"ph{b}_{m}")
                if USE_FP8_WO:
                    for th in range(TH):
                        for kc in range(DC // 2):
                            nc.tensor.matmul(
                                ph[:, th * 512:(th + 1) * 512],
                                lhsT=Wo_bf[:, 2 * kc:2 * kc + 2,
                                           m * P:(m + 1) * P],
                                rhs=silus[:, 2 * kc:2 * kc + 2,
                                          th * 512:(th + 1) * 512],
                                start=(kc == 0), stop=(kc == DC // 2 - 1),
                                perf_mode=PM_DR)
                else:
                    for th in range(TH):
                        for k in range(DC):
                            nc.tensor.matmul(
                                ph[:, th * 512:(th + 1) * 512],
                                lhsT=Wo_bf[:, k, m * P:(m + 1) * P],
                                rhs=silus[:, k, th * 512:(th + 1) * 512],
                                start=(k == 0), stop=(k == DC - 1))
                hc = hc_p.tile([P, T], BF16, tag="hc", name=f"hc{b}_{m}")
                nc.scalar.activation(out=hc[:], in_=ph[:], func=AF.Identity,
                                     bias=bo_t[:, m:m + 1])
                nc.scalar.dma_start_transpose(
                    out=hTt[:, :, m * P:(m + 1) * P], in_=hc[:])
            S[b, "hTt"] = hTt

        def phase_out(b):
            hTt = S[b, "hTt"]
            for hh in range(4):
                for tcn in range(hh * 2, (hh + 1) * 2):
                    nc.vector.tensor_add(out=hTt[:, tcn, :], in0=hTt[:, tcn, :],
                                         in1=xst[b][:, tcn, :])
                nc.gpsimd.dma_start(
                    out=out[b, hh * 256:(hh + 1) * 256, :]
                    .rearrange("(c p) d -> p c d", p=P),
                    in_=hTt[:, hh * 2:(hh + 1) * 2, :])

        phase_ln(0)
        emit_const_loads()
        phase_kv(0)
        for b in range(BPC):
            if b + 1 < BPC:
                load_batch(b + 1)
            phase_attn(b)
            phase_stats(b)
            if b + 1 < BPC:
                phase_ln(b + 1)
                phase_kv(b + 1)
            phase_zsilu(b)
            phase_wo(b)
            phase_out(b)
    return nc


def kernel(**inputs):
    global LAST_RESULT, LAST_NC
    x = np.asarray(inputs["x"], dtype=np.float32)
    xf = np.asarray(inputs["xf"], dtype=np.float32)
    emb = np.asarray(inputs["emb"], dtype=np.float32)
    cond = np.asarray(inputs["cond_type"]).reshape(B).astype(np.int64)
    gate = ((cond % 10) > 0).astype(np.float32).reshape(B, 1)

    wnames = ["Wq", "Wk", "Wv", "We", "Wo", "bq", "bk", "bv", "be", "bo",
              "ln_x_g", "ln_x_b", "ln_t_g", "ln_t_b", "ln_y_g", "ln_y_b"]
    w = {n: np.ascontiguousarray(np.asarray(inputs[n], dtype=np.float32))
         for n in wnames}

    # Host-side stylization precompute: silu(emb) @ We + be, folded with ln_y.
    sil = emb * (1.0 / (1.0 + np.exp(-emb)))
    eo = sil.astype(np.float32) @ w["We"] + w["be"]
    scale, shift = eo[:, :D], eo[:, D:]
    Asty = np.ascontiguousarray((w["ln_y_g"][None, :] * (1.0 + scale))
                                .astype(np.float32))
    Csty = np.ascontiguousarray((w["ln_y_b"][None, :] * (1.0 + scale) + shift)
                                .astype(np.float32))

    flags = {
        "bx": bool(np.any(w["ln_x_b"] != 0.0) or np.any(w["ln_x_g"] != 1.0)),
        "bt": bool(np.any(w["ln_t_b"] != 0.0) or np.any(w["ln_t_g"] != 1.0)),
        "bv": bool(np.any(w["bv"] != 0.0)),
        "bq": bool(np.any(w["bq"] != 0.0)),
        "bk": bool(np.any(w["bk"] != 0.0)),
    }

    nc = bacc.Bacc("TRN2", target_bir_lowering=False, debug=False,
                   enable_asserts=False, num_devices=NCORES)
    build(nc, flags)
    nc.compile()

    wdev = {n: w[n] for n in ["Wq", "Wk", "Wv", "Wo", "bq", "bk", "bv", "bo",
                              "ln_x_g", "ln_x_b", "ln_t_g", "ln_t_b"]}
    in_maps = []
    for i in range(NCORES):
        s = slice(i * BPC, (i + 1) * BPC)
        m = {"x": np.ascontiguousarray(x[s]),
             "xf": np.ascontiguousarray(xf[s]),
             "gate": np.ascontiguousarray(gate[s]),
             "Asty": np.ascontiguousarray(Asty[s]),
             "Csty": np.ascontiguousarray(Csty[s])}
        m.update(wdev)
        in_maps.append(m)

    LAST_NC = nc
    res = run_bass_kernel_spmd(nc, in_maps, core_ids=list(range(NCORES)))
    LAST_RESULT = res
    return np.concatenate([r["out"] for r in res.results], axis=0)
